# revision 42
# baseline (speedup 1.0000x reference)
"""Trainium2 Bass kernel for nn_IntegratedLaughterModel (v2).

Strategy (pure data parallel, 8 samples/core):
  - x sent as an fp8 pair (x8 = e4m3(x), r8 = e4m3(x - x8)): same bytes as
    bf16 but DoubleRow-matmul capable (157 TF/s) and ~2x more precise.
  - scores[b,h,s] = x[b,s,:] @ qk[:,h], qk = host-folded Wk@q_tom/sqrt(DH),
    scaled x256 into e4m3 range; exp applies 1/256.
  - xT for the scores matmul comes from PE transposes of the f32-VIEW of
    x8 (4 fp8 lanes per f32 element -> 4x fewer transposes); the resulting
    byte layout (d = 4*p + j) is consumed by DoubleRow matmuls with the
    j-dim as the k-tile pair axis.
  - 4 samples share each scores PSUM tile [44, 512] (rows 11j..11j+11);
    one sel-matmul adds the -32*256 mask penalty (attn rows) and the 0/-32*256
    log-mask rows (mean/setup/punch); one exp per group yields bf16 weights
    + f32 accumulated Z; weights are PE-transposed and cast to fp8 for the
    DoubleRow pooling matmuls over (x8, r8).
  - pooled rows (8 attn + 3 masked means, Z-normalized) -> feature-major
    head identical to the v1 kernel (bf16 weights, [128d, 8b] tiles).
"""

import os
import numpy as np

B, S, D, HID, NH = 64, 2048, 512, 512, 8
DH = D // NH
NCORES = 8
BPC = B // NCORES   # samples per core
NQ = 2              # quads per core
QS = 4              # samples per quad
NG = 4              # 512-token groups per sample
GT = 512            # tokens per group
KT = 16             # 128-token k-tiles per sample
NCD = 4             # d-chunks of 128
EPS = 1e-4
SCALE = 256.0       # qk prescale into e4m3 range
PEN = -32.0 * SCALE  # masked-token penalty (pre-exp-scale)

_CACHE = {}
LAST_RESULT = None


def _build_program():
    import concourse.bacc as bacc
    import concourse.tile as tile
    from concourse import mybir
    from contextlib import ExitStack

    f32 = mybir.dt.float32
    bf16 = mybir.dt.bfloat16
    fp8 = mybir.dt.float8e4
    AF = mybir.ActivationFunctionType
    ALU = mybir.AluOpType
    DR = mybir.MatmulPerfMode.DoubleRow

    nc = bacc.Bacc("TRN2", target_bir_lowering=False, debug=False,
                   enable_asserts=False)

    # ---- DRAM I/O ----
    # partition-major: x8[b, p, kt*D + d] = x8_natural[b, kt*128 + p, d]
    # -> each per-sample DMA is 128 contiguous 8KB descriptors
    x8_d = nc.dram_tensor("x8", [BPC, 128, KT * D], fp8, kind="ExternalInput").ap()
    r8_d = nc.dram_tensor("r8", [BPC, 128, KT * D], fp8, kind="ExternalInput").ap()
    qk8_d = nc.dram_tensor("qk8", [128, 4 * 4 * 64], fp8, kind="ExternalInput").ap()
    sel_d = nc.dram_tensor("sel64", [16, 64], bf16, kind="ExternalInput").ap()
    rows_d = nc.dram_tensor("rows16", [16, NQ * NG * GT], bf16,
                            kind="ExternalInput").ap()
    idf_d = nc.dram_tensor("identf", [128, 128], f32, kind="ExternalInput").ap()
    idb_d = nc.dram_tensor("identb", [128, 128], bf16, kind="ExternalInput").ap()
    id32_d = nc.dram_tensor("ident32", [16, 16], f32, kind="ExternalInput").ap()
    wv_d = nc.dram_tensor("wv", [128, 2048], bf16, kind="ExternalInput").ap()
    wtf_d = nc.dram_tensor("wtf", [128, 2048], bf16, kind="ExternalInput").ap()
    wg1_d = nc.dram_tensor("wg1", [128, 2048], bf16, kind="ExternalInput").ap()
    wg2_d = nc.dram_tensor("wg2", [128, 2048], bf16, kind="ExternalInput").ap()
    wc1_d = nc.dram_tensor("wc1", [128, 4096], bf16, kind="ExternalInput").ap()
    ws1_d = nc.dram_tensor("ws1", [128, 2048], bf16, kind="ExternalInput").ap()
    ws1t_d = nc.dram_tensor("ws1t", [3, 512], bf16, kind="ExternalInput").ap()
    wf1_d = nc.dram_tensor("wf1", [128, 2048], bf16, kind="ExternalInput").ap()
    wf1t_d = nc.dram_tensor("wf1t", [3, 512], bf16, kind="ExternalInput").ap()
    vecs_d = nc.dram_tensor("vecs", [128, 20], bf16, kind="ExternalInput").ap()
    bvecs_d = nc.dram_tensor("bvecs", [128, 20], f32, kind="ExternalInput").ap()
    b5_d = nc.dram_tensor("b5", [1, 5], f32, kind="ExternalInput").ap()
    m3_d = nc.dram_tensor("m3", [1, 9], f32, kind="ExternalInput").ap()
    out_d = nc.dram_tensor("out", [1, BPC], f32, kind="ExternalOutput").ap()
    diag_d = nc.dram_tensor("diag", [BPC * 11, D], bf16, kind="ExternalOutput").ap()

    with tile.TileContext(nc) as tc, ExitStack() as ctx:
        cst = ctx.enter_context(tc.tile_pool(name="cst", bufs=1))

        def static(name, shape, src, dt=f32):
            t = cst.tile(shape, dt, tag=name, name=name)
            nc.sync.dma_start(out=t[:], in_=src)
            return t

        def static_g(name, shape, src, dt=f32):
            t = cst.tile(shape, dt, tag=name, name=name)
            nc.gpsimd.dma_start(out=t[:], in_=src)
            return t

        ones_sb = cst.tile([128, 1], f32, tag="ones")
        nc.vector.memset(ones_sb[:], 1.0)
        onesrow = cst.tile([1, 128], f32, tag="onesrow")
        nc.vector.memset(onesrow[:], 1.0 / 3.0)

        # pooledT: [128 d, c-chunk x sample x quantity] feature-major pooled
        pTall = cst.tile([128, NCD * BPC * 11], bf16, tag="pTall", name="pTall")

        H = {}

        def static_s(name, shape, src, dt=f32):
            t = cst.tile(shape, dt, tag=name, name=name)
            nc.scalar.dma_start(out=t[:], in_=src)
            return t

        def load_head_weights(tranche):
            if tranche == 0:
                H["wv"] = static("wv", [128, 2048], wv_d, bf16)
                H["wtf"] = static("wtf", [128, 2048], wtf_d, bf16)
                H["vecs"] = static_g("vecs", [128, 20], vecs_d, bf16)
                H["bvecs"] = static_g("bvecs", [128, 20], bvecs_d)
                H["b5"] = static_g("b5", [1, 5], b5_d)
                H["m3"] = static_g("m3", [1, 9], m3_d)
            elif tranche == 1:
                H["wg1"] = static("wg1", [128, 2048], wg1_d, bf16)
                H["wg2"] = static("wg2", [128, 2048], wg2_d, bf16)
                H["wc1"] = static("wc1", [128, 4096], wc1_d, bf16)
            else:
                H["ws1"] = static("ws1", [128, 2048], ws1_d, bf16)
                H["ws1t"] = static("ws1t", [3, 512], ws1t_d, bf16)
                H["wf1"] = static("wf1", [128, 2048], wf1_d, bf16)
                H["wf1t"] = static("wf1t", [3, 512], wf1t_d, bf16)

        # ================= main streaming pass =================
        with ExitStack() as pctx:
            x8_p = pctx.enter_context(tc.tile_pool(name="x8", bufs=6))
            r8_p = pctx.enter_context(tc.tile_pool(name="r8", bufs=6))
            xt_ps_p = pctx.enter_context(tc.tile_pool(name="xtps", bufs=2,
                                                      space="PSUM"))
            xt_sb_p = pctx.enter_context(tc.tile_pool(name="xtsb", bufs=5))
            sc_ps_p = pctx.enter_context(tc.tile_pool(name="scps", bufs=2,
                                                      space="PSUM"))
            w8_p = pctx.enter_context(tc.tile_pool(name="w8", bufs=2))
            wt_ps_p = pctx.enter_context(tc.tile_pool(name="wtps", bufs=2,
                                                      space="PSUM"))
            wt_sb_p = pctx.enter_context(tc.tile_pool(name="wtsb", bufs=2))
            pool_ps_p = pctx.enter_context(tc.tile_pool(name="poolps", bufs=2,
                                                        space="PSUM"))
            small_p = pctx.enter_context(tc.tile_pool(name="small", bufs=4))

            def fetch(pool, dram, b, name, eng, nsplit=2):
                # split DMAs so consumers unblock at sub-sample granularity;
                # all on the (otherwise idle) sync queue
                t = pool.tile([128, KT * D], fp8, tag=name, name=f"{name}{b}")
                hw = KT * D // nsplit
                for tp in range(nsplit):
                    nc.sync.dma_start(out=t[:, tp * hw:(tp + 1) * hw],
                                      in_=dram[b][:, tp * hw:(tp + 1) * hw])
                return t

            # HWDGE queues are FIFO per engine: small constants first (the
            # first PE transpose needs identf), then 6 samples of x data,
            # then (interleaved with quad emission) the head weights.
            # sync-queue order: identf then the first x halves (the first
            # transposes need exactly these), remaining constants threaded in
            pre = {}
            idf_sb = static("identf", [128, 128], idf_d, f32)
            for b in range(2):
                pre[("x", b)] = fetch(x8_p, x8_d, b, "x8s", None, nsplit=4)
            qk8_sb = static("qk8", [128, 4 * 4 * 64], qk8_d, fp8)
            sel_sb = static("sel64", [16, 64], sel_d, bf16)
            for b in range(2, 6):
                pre[("x", b)] = fetch(x8_p, x8_d, b, "x8s", None)
            idb_sb = static("identb", [128, 128], idb_d, bf16)
            id32_sb = static("ident32", [16, 16], id32_d, f32)
            rows_sb = static("rows16", [16, NQ * NG * GT], rows_d, bf16)
            for b in range(6):
                pre[("r", b)] = fetch(r8_p, r8_d, b, "r8s", None)

            def emit_quad(q):
                w8q = w8_p.tile([64, NG * GT], bf16, tag="w8", name=f"w8_{q}")
                zcq = small_p.tile([64, NG], f32, tag="zc", name=f"zc{q}")
                wT8 = wt_sb_p.tile([128, KT * 64], fp8, tag="wT",
                                   name=f"wT{q}")
pass
                x8s, r8s, xtq = [], [], []
                for j in range(QS):
                    b = QS * q + j
                    if ("x", b) in pre:
                        x8s.append(pre[("x", b)])
                        r8s.append(pre[("r", b)])
                    else:
                        x8s.append(fetch(x8_p, x8_d, b, "x8s", None))
                        r8s.append(fetch(r8_p, r8_d, b, "r8s", None))
                # transposes of the f32 view: d = 4*p + jj after transpose.
                # Interleave group-0 scores between per-sample transposes so
                # the (in-order) PE queue has ready work while later samples'
                # x data is still arriving.
                qkv = qk8_sb[:].rearrange(
                    "p (j jj c) -> p j jj c", j=4, jj=4)
                sc_tiles = {}

                def emit_tr(j):
                    xt = xt_sb_p.tile([128, KT * 128], f32, tag="xt",
                                      name=f"xt{q}_{j}")
                    xv = x8s[j][:].bitcast(f32).rearrange(
                        "p (kt t) -> p kt t", t=128)
                    for tp in range(4):
                        ps = xt_ps_p.tile([128, 512], f32, tag="xtp",
                                          name=f"xtp{q}_{j}_{tp}")
                        for k in range(4):
                            nc.tensor.transpose(
                                ps[:, k * 128:(k + 1) * 128],
                                xv[:, 4 * tp + k, :], idf_sb[:])
                        dst = xt[:, tp * 512:(tp + 1) * 512]
                        if tp % 2 == 0:
                            nc.vector.tensor_copy(dst, ps[:])
                        else:
                            nc.scalar.copy(dst, ps[:])
                    xtq.append(xt)

                def emit_sc(g, j):
                    if g not in sc_tiles:
                        sc_tiles[g] = sc_ps_p.tile([64, GT], f32, tag="sc",
                                                   name=f"sc{q}_{g}")
                    sc_ps = sc_tiles[g]
                    xt8 = xtq[j][:].bitcast(fp8).rearrange(
                        "p (t jj) -> p jj t", jj=4)
                    for i in range(2):
                        nc.tensor.matmul(
                            sc_ps[:, :],
                            qkv[:, j, 2 * i:2 * i + 2, :],
                            xt8[:, 2 * i:2 * i + 2, g * GT:(g + 1) * GT],
                            start=(j == 0 and i == 0), stop=False,
                            perf_mode=DR)

                emit_tr(0)
                emit_tr(1)
                emit_sc(0, 0)
                emit_sc(0, 1)
                emit_tr(2)
                emit_sc(0, 2)
                emit_tr(3)
                emit_sc(0, 3)

                for g in range(NG):
                    if g > 0:
                        for j in range(QS):
                            emit_sc(g, j)
                    sc_ps = sc_tiles[g]
                    nc.tensor.matmul(
                        sc_ps[:, :], sel_sb[:],
                        rows_sb[:, (q * NG + g) * GT:(q * NG + g + 1) * GT],
                        start=False, stop=True)
                    nc.scalar.activation(w8q[:, g * GT:(g + 1) * GT], sc_ps[:],
                                         AF.Exp, scale=1.0 / SCALE,
                                         accum_out=zcq[:, g:g + 1])

                # weight transposes emitted after ALL scores: the PE queue is
                # in-order, and a wT transpose waiting on its exp (ACT) would
                # head-of-line-block the next group's scores matmuls
                for g in range(NG):
                    for k in range(4):
                        wt_ps = wt_ps_p.tile([128, 128], bf16, tag="wtp",
                                             name=f"wtp{q}_{g}_{k}")
                        nc.tensor.transpose(
                            wt_ps[:, 0:64],
                            w8q[:, g * GT + k * 128: g * GT + (k + 1) * 128],
                            idb_sb[:64, :64])
                        nc.vector.tensor_copy(
                            wT8[:, (4 * g + k) * 64:(4 * g + k + 1) * 64],
                            wt_ps[:, 0:64])

                # pooling: 16 DoubleRow matmuls per sample (x8 then r8);
                # DR dst must start at partition 0 -> one [16, D] tile each
                wTv = wT8[:].rearrange("p (kt c) -> p kt c", c=64)
                pool_t = []
                for j in range(QS):
                    pr = pool_ps_p.tile([16, D], f32, tag="pool",
                                        name=f"pool{q}_{j}")
                    srcs = (x8s[j], r8s[j])
                    for si, src in enumerate(srcs):
                        sv = src[:].rearrange("p (kt d) -> p kt d", d=D)
                        for kp in range(KT // 2):
                            nc.tensor.matmul(
                                pr[:, :],
                                wTv[:, 2 * kp:2 * kp + 2, 16 * j:16 * j + 16],
                                sv[:, 2 * kp:2 * kp + 2, :],
                                start=(si == 0 and kp == 0),
                                stop=(si == 1 and kp == KT // 2 - 1),
                                perf_mode=DR)
                    pool_t.append(pr)

                # Z as a broadcast row: z1 [44,1] -> transpose -> 1/Z -> [128,44]
                z1 = small_p.tile([64, 1], f32, tag="z1", name=f"z1_{q}")
                nc.vector.tensor_reduce(z1[:], zcq[:],
                                        mybir.AxisListType.X, ALU.add)
                zrow_ps = wt_ps_p.tile([128, 128], bf16, tag="wtp",
                                       name=f"zrow{q}").bitcast(f32)[0:1, 0:64]
                nc.tensor.transpose(zrow_ps, z1[:], idf_sb[:64, :64])
                zrow_sb = small_p.tile([1, 64], f32, tag="zrow",
                                       name=f"zrs{q}")
                nc.vector.reciprocal(zrow_sb[:], zrow_ps)
                bc64 = small_p.tile([128, 64], f32, tag="bc64",
                                    name=f"bc{q}")
                nc.gpsimd.partition_broadcast(bc64[:], zrow_sb[:])

                # evacuate pooled pairs (unnormalized) and extract pooledT
                # into pTall, applying 1/Z as a per-column multiply
                for j in range(QS):
                    pooled_sb = small_p.tile([16, D], bf16, tag="pooled",
                                             name=f"pl{q}_{j}")
                    nc.scalar.activation(pooled_sb[:], pool_t[j][:], AF.Copy)
                    for c in range(NCD):
                        pt_ps = wt_ps_p.tile([128, 128], bf16, tag="wtp",
                                             name=f"ptp{q}_{j}_{c}")
                        nc.tensor.transpose(pt_ps[:, 0:16],
                                            pooled_sb[:, c * 128:(c + 1) * 128],
                                            idb_sb[:16, :16])
                        nc.vector.tensor_mul(
                            pTall[:, c * 88 + 44 * q + 11 * j:
                                  c * 88 + 44 * q + 11 * j + 11],
                            pt_ps[:, 0:11],
                            bc64[:, 16 * j:16 * j + 11])

            emit_quad(0)
            load_head_weights(0)
            load_head_weights(1)
            emit_quad(1)
            load_head_weights(2)

        # ================= head (feature-major, all 8 samples) =================
        def cview(c, r):
            """[128, 8] view of quantity r across samples in pooledT chunk c."""
            return pTall[:].rearrange("p (c b q) -> p c b q", b=BPC, q=11)[
                :, c, :, r]

        with ExitStack() as hctx:
            pj = hctx.enter_context(tc.tile_pool(name="pj", bufs=4, space="PSUM"))
            ptiny = hctx.enter_context(tc.tile_pool(name="ptiny", bufs=1, space="PSUM"))
            hp = hctx.enter_context(tc.tile_pool(name="hp", bufs=1))
            htmp = hctx.enter_context(tc.tile_pool(name="htmp", bufs=4))

            wv = H["wv"]; wtf = H["wtf"]; wg1 = H["wg1"]; wg2 = H["wg2"]
            wc1 = H["wc1"]; ws1 = H["ws1"]; ws1t = H["ws1t"]; wf1 = H["wf1"]
            wf1t = H["wf1t"]; vecs = H["vecs"]; bvecs = H["bvecs"]
            b5 = H["b5"]; m3 = H["m3"]

            def vcol(k, c):
                return vecs[:, k * 4 + c: k * 4 + c + 1]

            def bcol(k, c):
                return bvecs[:, k * 4 + c: k * 4 + c + 1]

            # streams tile: col = c*24 + i*8 + b  (i: 0=fm, 1=ctx, 2=clost)
            str_sb = hp.tile([128, 96], bf16, tag="strs")

            def sview(i, c=None):
                v = str_sb[:].rearrange("p (c i b) -> p i c b", i=3, b=BPC)
                return v[:, i, :, :] if c is None else v[:, i, c, :]

            # ---- fusedT = blockdiag(Wv) applied to attn-pooled heads ----
            fused_sb = []
            for i in range(4):
                ps = pj.tile([128, BPC], f32, tag="proj")
                for hh in range(2):
                    h = 2 * i + hh
                    o = ps[hh * 64:(hh + 1) * 64, :]
                    for c in range(NCD):
                        nc.tensor.matmul(
                            o, wv[:, c * D + h * DH: c * D + (h + 1) * DH],
                            cview(c, h), start=(c == 0), stop=(c == NCD - 1))
                t = hp.tile([128, BPC], bf16, tag=f"fused{i}")
                nc.vector.tensor_copy(t[:], ps[:])
                fused_sb.append(t)

            def proj512(w_tile, rhs_aps, consume, nchunks=4):
                outs = []
                for jc in range(4):
                    ps = pj.tile([128, BPC], f32, tag="proj")
                    for c in range(nchunks):
                        nc.tensor.matmul(
                            ps[:],
                            w_tile[:, c * D + jc * 128: c * D + jc * 128 + 128],
                            rhs_aps[c], start=(c == 0), stop=(c == nchunks - 1))
                    outs.append(consume(jc, ps))
                return outs

            def copy_out(tagp):
                def f(jc, ps):
                    t = hp.tile([128, BPC], bf16, tag=f"{tagp}{jc}")
                    nc.vector.tensor_copy(t[:], ps[:])
                    return t
                return f

            def relu_out(tagp, bk):
                def f(jc, ps):
                    t = hp.tile([128, BPC], bf16, tag=f"{tagp}{jc}")
                    nc.scalar.activation(t[:], ps[:], AF.Relu, bias=bcol(bk, jc))
                    return t
                return f

            # ---- fused_mental (stream 0); tom_hp pre-sigmoid ----
            def into_stream(i, bk=None):
                def f(jc, ps):
                    if bk is None:
                        nc.vector.tensor_copy(sview(i, jc), ps[:])
                    else:
                        nc.vector.tensor_scalar_add(sview(i, jc), ps[:],
                                                    bcol(bk, jc))
                    return None
                return f

            proj512(wtf, [t[:] for t in fused_sb], into_stream(0))
            s3_ps = ptiny.tile([1, 24], f32, tag="s3")
            for c in range(4):
                nc.tensor.matmul(s3_ps[:, 0:8], vcol(0, c), sview(0, c),
                                 start=(c == 0), stop=(c == 3))

            # ---- GCACU (ctx with bg2 folded in = stream 1) ----
            h1_sb = proj512(wg1, [cview(c, 8) for c in range(4)], relu_out("h1", 0))
            proj512(wg2, [t[:] for t in h1_sb], into_stream(1, bk=1))
            for c in range(4):
                nc.tensor.matmul(s3_ps[:, 8:16], vcol(1, c), sview(1, c),
                                 start=(c == 0), stop=(c == 3))

            # ---- CLoST ----
            c1_sb = []
            for jc in range(4):
                ps = pj.tile([128, BPC], f32, tag="proj")
                for cc in range(8):
                    rhs = cview(cc, 9) if cc < 4 else cview(cc - 4, 10)
                    nc.tensor.matmul(
                        ps[:], wc1[:, cc * 512 + jc * 128: cc * 512 + jc * 128 + 128],
                        rhs, start=(cc == 0), stop=(cc == 7))
                t = hp.tile([128, BPC], bf16, tag=f"hc{jc}")
                nc.scalar.activation(t[:], ps[:], AF.Relu, bias=bcol(2, jc))
                c1_sb.append(t)
            for c in range(4):
                nc.tensor.matmul(s3_ps[:, 16:24], vcol(2, c), c1_sb[c][:],
                                 start=(c == 0), stop=(c == 3))
            # stream 2 = setup+punch (0.5 folded into m3 col 2 on host)
            pv = pTall[:].rearrange("p (c b q) -> p q c b", b=BPC, q=11)
            nc.vector.tensor_add(sview(2), pv[:, 9, :, :], pv[:, 10, :, :])

            # ---- scores3: add scalar biases, sigmoid ----
            s3b_sb = hp.tile([1, 24], f32, tag="s3b")
            nc.vector.tensor_scalar_add(s3b_sb[:, 0:8], s3_ps[:, 0:8], b5[:, 0:1])
            nc.vector.tensor_scalar_add(s3b_sb[:, 8:16], s3_ps[:, 8:16], b5[:, 1:2])
            nc.vector.tensor_scalar_add(s3b_sb[:, 16:24], s3_ps[:, 16:24], b5[:, 2:3])
            s3_sb = hp.tile([1, 24], f32, tag="s3s")
            nc.scalar.activation(s3_sb[:], s3b_sb[:], AF.Sigmoid)

            # scores3T [3, 8] via double transpose
            sbt_ps = pj.tile([128, BPC], f32, tag="proj")
            for t in range(3):
                nc.tensor.transpose(sbt_ps[0:8, t:t + 1],
                                    s3_sb[:, t * 8:(t + 1) * 8], id32_sb[:1, :1])
            sbt_sb = hp.tile([8, 3], f32, tag="sbt")
            nc.vector.tensor_copy(sbt_sb[:], sbt_ps[0:8, 0:3])
            s3t_ps = pj.tile([128, BPC], f32, tag="proj")
            nc.tensor.transpose(s3t_ps[0:3, 0:8], sbt_sb[:], id32_sb[:8, :8])
            s3t_sb = hp.tile([3, 8], bf16, tag="s3t")
            nc.vector.tensor_copy(s3t_sb[:], s3t_ps[0:3, 0:8])

            # ---- mHC mix + unit-norm + mean over streams (wide tiles) ----
            m3bc = hp.tile([128, 9], f32, tag="m3bc")
            nc.gpsimd.partition_broadcast(m3bc[:], m3[:])
            mx_sb = hp.tile([128, 96], bf16, tag="mx")  # col = i*32 + c*8 + b
            for i in range(3):
                # fresh temps per stream: reusing one tile WAR-serializes
                # the three otherwise-independent mixing chains
                tA = htmp.tile([128, 32], bf16, tag="mxa")
                tB = htmp.tile([128, 32], bf16, tag="mxb")
                mv = mx_sb[:, i * 32:(i + 1) * 32]
                nc.vector.tensor_scalar_mul(tA[:], sview(0),
                                            m3bc[:, i * 3:i * 3 + 1])
                nc.vector.scalar_tensor_tensor(
                    tB[:], sview(1), m3bc[:, i * 3 + 1:i * 3 + 2],
                    tA[:], ALU.mult, ALU.add)
                nc.vector.scalar_tensor_tensor(
                    mv, sview(2), m3bc[:, i * 3 + 2:i * 3 + 3],
                    tB[:], ALU.mult, ALU.add)
            sq = htmp.tile([128, 96], f32, tag="sq")
            nc.vector.tensor_mul(sq[:], mx_sb[:], mx_sb[:])
            ss_ps = ptiny.tile([1, 96], f32, tag="ss")
            nc.tensor.matmul(ss_ps[:], ones_sb[:], sq[:], start=True, stop=True)
            ss24 = hp.tile([1, 24], f32, tag="ss24")
            nc.vector.tensor_reduce(
                ss24[:].rearrange("p (i b) -> p i b", i=3),
                ss_ps[:].rearrange("p (i c b) -> p i b c", i=3, c=4),
                mybir.AxisListType.X, ALU.add)
            nrm_sb = hp.tile([1, 24], f32, tag="nrm")
            nc.scalar.activation(nrm_sb[:], ss24[:], AF.Sqrt)
            inv_sb = hp.tile([1, 24], f32, tag="inv")
            nc.vector.reciprocal(inv_sb[:], nrm_sb[:])
            invbc = ptiny.tile([128, 24], f32, tag="bc")
            nc.tensor.matmul(invbc[:], onesrow[:], inv_sb[:],
                             start=True, stop=True)

            def mxv(i):
                return mx_sb[:, i * 32:(i + 1) * 32].rearrange(
                    "p (c b) -> p c b", b=BPC)

            def invv(i):
                return invbc[:, i * 8:(i + 1) * 8].rearrange(
                    "p (one b) -> p one b", one=1).broadcast_to([128, 4, BPC])

            pm_sb = hp.tile([128, 32], bf16, tag="pmix")  # col = c*8 + b
            pmv = pm_sb[:].rearrange("p (c b) -> p c b", b=BPC)
            t0 = htmp.tile([128, 32], bf16, tag="pm0")
            t1 = htmp.tile([128, 32], bf16, tag="pm1")
            t0v = t0[:].rearrange("p (c b) -> p c b", b=BPC)
            t1v = t1[:].rearrange("p (c b) -> p c b", b=BPC)
            nc.vector.tensor_mul(t0v, mxv(0), invv(0))
            nc.vector.tensor_mul(t1v, mxv(1), invv(1))
            nc.vector.tensor_add(t0v, t0v, t1v)
            nc.vector.tensor_mul(t1v, mxv(2), invv(2))
            nc.vector.tensor_add(pmv, t0v, t1v)
            pmix_sb = [pm_sb[:, c * 8:(c + 1) * 8] for c in range(4)]

            # ---- SEVADE + final head ----
            fin_ps = ptiny.tile([1, 16], f32, tag="fin")
            for (w_main, w_tail, vk, bk, col) in (
                    (ws1, ws1t, 3, 3, 0), (wf1, wf1t, 4, 4, 8)):
                for jc in range(4):
                    ps = pj.tile([128, BPC], f32, tag="proj")
                    for c in range(4):
                        nc.tensor.matmul(
                            ps[:],
                            w_main[:, c * D + jc * 128: c * D + jc * 128 + 128],
                            pmix_sb[c], start=(c == 0), stop=False)
                    nc.tensor.matmul(ps[:], w_tail[:, jc * 128: jc * 128 + 128],
                                     s3t_sb[:], start=False, stop=True)
                    hs = htmp.tile([128, BPC], bf16, tag="hs")
                    nc.scalar.activation(hs[:], ps[:], AF.Relu, bias=bcol(bk, jc))
                    nc.tensor.matmul(fin_ps[:, col:col + 8], vcol(vk, jc), hs[:],
                                     start=(jc == 0), stop=(jc == 3))

            # ---- combine: fin + 0.5*sev + 0.1*safe_logit(mean(s3)) ----
            sev_l = hp.tile([1, 8], f32, tag="sevl")
            nc.vector.tensor_scalar_add(sev_l[:], fin_ps[:, 0:8], b5[:, 3:4])
            fin_l = hp.tile([1, 8], f32, tag="finl")
            nc.vector.tensor_scalar_add(fin_l[:], fin_ps[:, 8:16], b5[:, 4:5])
            t1 = hp.tile([1, 8], f32, tag="t1")
            nc.vector.tensor_add(t1[:], s3_sb[:, 0:8], s3_sb[:, 8:16])
            t2 = hp.tile([1, 8], f32, tag="t2")
            nc.vector.tensor_add(t2[:], t1[:], s3_sb[:, 16:24])
            pm3 = hp.tile([1, 8], f32, tag="pm3")
            nc.vector.tensor_scalar_mul(pm3[:], t2[:], 1.0 / 3.0)
            pcl = hp.tile([1, 8], f32, tag="pcl")
            nc.vector.tensor_scalar(pcl[:], pm3[:], EPS, 1.0 - EPS,
                                    ALU.max, ALU.min)
            lp = hp.tile([1, 8], f32, tag="lp")
            nc.scalar.activation(lp[:], pcl[:], AF.Ln)
            omp = hp.tile([1, 8], f32, tag="omp")
            nc.vector.tensor_scalar(omp[:], pcl[:], -1.0, 1.0, ALU.mult, ALU.add)
            l1p = hp.tile([1, 8], f32, tag="l1p")
            nc.scalar.activation(l1p[:], omp[:], AF.Ln)
            lg = hp.tile([1, 8], f32, tag="lg")
            nc.vector.tensor_sub(lg[:], lp[:], l1p[:])
            o1 = hp.tile([1, 8], f32, tag="o1")
            nc.vector.scalar_tensor_tensor(o1[:], sev_l[:], 0.5, fin_l[:],
                                           ALU.mult, ALU.add)
            o2 = hp.tile([1, 8], f32, tag="o2")
            nc.vector.scalar_tensor_tensor(o2[:], lg[:], 0.1, o1[:],
                                           ALU.mult, ALU.add)
            nc.gpsimd.dma_start(out=out_d[:], in_=o2[:])

    nc.compile()
    return nc


def _pack_w(w, ncol=512):
    w = np.asarray(w, np.float32)
    nchunk = w.shape[0] // 128
    return np.ascontiguousarray(
        w.reshape(nchunk, 128, ncol).transpose(1, 0, 2).reshape(128, nchunk * ncol))


def _pack_v(v):
    v = np.asarray(v, np.float32).reshape(-1)
    return np.ascontiguousarray(v.reshape(4, 128).T)


def _prep_host(inputs):
    import ml_dtypes
    bf = ml_dtypes.bfloat16
    e4 = ml_dtypes.float8_e4m3
    f8 = np.float64

    def q8(a):
        return np.clip(np.asarray(a, np.float32), -240.0, 240.0).astype(e4)

    Wk = np.asarray(inputs["Wk"], f8)
    q_tom = np.asarray(inputs["q_tom"], f8)
    qk = np.einsum("dhk,hk->dh", Wk.reshape(D, NH, DH), q_tom) / np.sqrt(
        np.float64(DH))
    # qk8big[p, j, jj, 16j+h] = SCALE * qk[4p + jj, h]; other cols zero.
    # (zero-padded per-sample stationary so the scores matmul can write the
    # full dense [64, 512] PSUM tile: matmul cost depends only on N; M must
    # be a multiple of 16 for dual-fp8 LDWEIGHTS, rows are 16-packed)
    qk_s = np.zeros((128, 4, 11), np.float32)
    qk_s[:, :, :NH] = (SCALE * qk).astype(np.float32).reshape(128, 4, NH)
    qk8big = np.zeros((128, 4, 4, 64), np.float32)
    for j in range(QS):
        qk8big[:, j, :, 16 * j:16 * j + 11] = qk_s
    qk8 = q8(qk8big.reshape(128, 4 * 4 * 64))

    sel = np.zeros((16, 64), np.float32)
    for j in range(QS):
        for h in range(8):
            sel[4 * j, 16 * j + h] = 1.0
        for t in range(3):
            sel[4 * j + 1 + t, 16 * j + 8 + t] = 1.0
    sel = sel.astype(bf)

    m = np.asarray(inputs["attention_mask"], f8)  # [B, S]
    cum = np.cumsum(m, axis=1)
    valid = cum[:, -1:]
    split = np.maximum(1.0, np.floor(valid * 0.6))
    setup = m * (cum <= split)
    punch = m * (cum > split)
    pc = punch.sum(1, keepdims=True)
    last = m * (cum == valid)
    punch = np.where(pc > 0, punch, last)
    # rows16 [B -> (core, quad, j), 4 rows, S]: PEN * (1 - mask)
    masks = np.stack([m, m, setup, punch], 1)  # [B, 4, S]
    rows = (PEN * (1.0 - masks)).astype(np.float32).astype(bf)  # [B,4,S]

    M3 = (np.eye(3, dtype=f8)
          + np.asarray(inputs["U_mhc"], f8) @ np.asarray(inputs["V_mhc"], f8))
    M3 = M3.copy()
    M3[:, 2] *= 0.5  # clost stream sent as raw setup+punch
    m3 = np.ascontiguousarray(M3.astype(np.float32).reshape(1, 9))

    Ws1 = np.asarray(inputs["Ws1"], np.float32)
    Wf1 = np.asarray(inputs["Wf1"], np.float32)
    vecs = np.concatenate([
        _pack_v(inputs["w_hp"]), _pack_v(inputs["w_inc"]), _pack_v(inputs["wc2"]),
        _pack_v(inputs["ws2"]), _pack_v(inputs["wf2"])], axis=1)
    bvecs = np.concatenate([
        _pack_v(inputs["bg1"]), _pack_v(inputs["bg2"]), _pack_v(inputs["bc1"]),
        _pack_v(inputs["bs1"]), _pack_v(inputs["bf1"])], axis=1)
    b5 = np.array([[np.float32(np.asarray(inputs[k]).reshape(-1)[0])
                    for k in ("b_hp", "b_inc", "bc2", "bs2", "bf2")]], np.float32)

    shared = {
        "qk8": qk8, "sel64": sel,
        "identf": np.eye(128, dtype=np.float32),
        "identb": np.eye(128, dtype=np.float32).astype(bf),
        "ident32": np.eye(16, dtype=np.float32),
        "wv": _pack_w(inputs["Wv"]).astype(bf),
        "wtf": _pack_w(inputs["W_tom_fuse"]).astype(bf),
        "wg1": _pack_w(inputs["Wg1"]).astype(bf),
        "wg2": _pack_w(inputs["Wg2"]).astype(bf),
        "wc1": _pack_w(inputs["Wc1"]).astype(bf),
        "ws1": _pack_w(Ws1[:512]).astype(bf),
        "ws1t": np.ascontiguousarray(Ws1[512:515]).astype(bf),
        "wf1": _pack_w(Wf1[:512]).astype(bf),
        "wf1t": np.ascontiguousarray(Wf1[512:515]).astype(bf),
        "vecs": np.ascontiguousarray(vecs).astype(bf),
        "bvecs": np.ascontiguousarray(bvecs),
        "b5": b5, "m3": m3,
    }
    x = np.asarray(inputs["embeddings"], np.float32)
    x8 = q8(x)
    r8 = q8(x - x8.astype(np.float32))
    # partition-major shuffle: [B, S, D] -> [B, p(128), kt(16)*D]
    x8 = np.ascontiguousarray(
        x8.reshape(B, KT, 128, D).transpose(0, 2, 1, 3).reshape(B, 128, KT * D))
    r8 = np.ascontiguousarray(
        r8.reshape(B, KT, 128, D).transpose(0, 2, 1, 3).reshape(B, 128, KT * D))
    in_maps = []
    for k in range(NCORES):
        d = dict(shared)
        d["x8"] = x8[k * BPC:(k + 1) * BPC]
        d["r8"] = r8[k * BPC:(k + 1) * BPC]
        d["rows16"] = np.ascontiguousarray(
            rows[k * BPC:(k + 1) * BPC].reshape(NQ, 16, S)
            .transpose(1, 0, 2).reshape(16, NQ * S))
        in_maps.append(d)
    return in_maps


def _install_ntff_shim():
    """antenv.axon_hooks is absent in this image; recreate it so
    run_bass_kernel_spmd(trace=True) can capture NTFF profiles."""
    import sys
    import types
    if "antenv.axon_hooks" in sys.modules:
        return
    mod = types.ModuleType("antenv.axon_hooks")
    mod._hook = None
    mod.set_axon_ntff_profile_hook = lambda h: setattr(mod, "_hook", h)
    mod.get_axon_ntff_profile_hook = lambda: mod._hook
    sys.modules["antenv.axon_hooks"] = mod
    try:
        import antenv
        antenv.axon_hooks = mod
        from trn_agent_boot.trn_boot import _ntff_profile_via_ctypes
        mod._hook = _ntff_profile_via_ctypes("/opt/axon/libaxon_pjrt.so")
    except Exception as e:
        print(f"ntff shim setup failed ({e}); tracing disabled")


def kernel(**inputs):
    global LAST_RESULT
    _install_ntff_shim()
    from concourse.bass_utils import run_bass_kernel_spmd

    if "nc" not in _CACHE:
        _CACHE["nc"] = _build_program()
    nc = _CACHE["nc"]

    in_maps = _prep_host(inputs)
    trace = os.environ.get("BASS_TRACE", "0") == "1"
    res = run_bass_kernel_spmd(nc, in_maps, list(range(NCORES)), trace=trace)
    LAST_RESULT = res
    out = np.empty((B, 1), np.float32)
    for k in range(NCORES):
        out[k * BPC:(k + 1) * BPC, 0] = np.asarray(res.results[k]["out"]).reshape(-1)
    return out
